# revision 2
# baseline (speedup 1.0000x reference)
"""BKT kernel v2: j-major layout + binary product tree + PE collapse.

Layout per core: 128 partitions x 1536 free. Row map: partition p<64 ->
(student b=p, abilities {0,1,2} in slabs r=0,1,2); p>=64 -> (b=p-64,
abilities {3,4} in slabs 0,1; slab 2 pad). Free index f = j*192 + jb*24 +
r*8 + sb where t = 64*sb + 8*jb + j.  All heavy TTs are innermost-packed.
Ability collapse: per-group max (in-lane slab maxes) + one partition-swap
DMA + PE matmuls with 0/1 masks into PSUM.
"""

import os
import numpy as np
from contextlib import ExitStack

import concourse.bass as bass
import concourse.bacc as bacc
import concourse.mybir as mybir
from concourse import tile
from concourse.bass_utils import run_bass_kernel_spmd

F32 = mybir.dt.float32
BF16 = mybir.dt.bfloat16
Alu = mybir.AluOpType
Act = mybir.ActivationFunctionType
AX = mybir.AxisListType

N_CORES = 8
T = 512
BL = 64                    # students per core
J = 8                      # steps per block
NB = 64                    # blocks per slab
SBN = 8                    # blocks per super
NSUP = 8                   # supers per slab
RT = 3                     # slabs
NS = RT * SBN              # 24: (r, sb) merged
NRB = 8 * NS               # 192: (jb, r, sb) merged
F = J * NRB                # 1536

# dtype knobs (env KV2_CFG: letters t=tree, a=alphas, p=preds, g=G/S -> bf16)
_cfg = os.environ.get("KV2_CFG", "tag")
DT_TREE = BF16 if "t" in _cfg else F32
DT_AL = BF16 if "a" in _cfg else F32
DT_PRED = BF16 if "p" in _cfg else F32
DT_GS = BF16 if "g" in _cfg else F32

_last_results = None
_cached_nc = None
DEBUG = bool(os.environ.get("KV2_DEBUG"))
_DBG_SHAPES = {"D_M": 4*1536, "D_Q": 4*192, "D_SB": 4*24, "D_AB": 2*192,
               "D_AL": 2*1536, "D_AP": 3*513, "D_MO": 512,
               "D_UE": 512}


def _ap(base, off, dims):
    return bass.AP(base.tensor, base.offset + off, [list(base.ap[0])] + dims)


def _ap_p(base, poff, pcount, off, dims):
    p = list(base.ap[0])
    pstride = p[0]
    return bass.AP(base.tensor, base.offset + poff * pstride + off,
                   [[pstride, pcount]] + dims)


def _emit(ctx, tc, nc, G, S, C, Y, K, WM, O, dbg=None):
    v = nc.vector
    sc = nc.scalar
    sy = nc.sync
    g = nc.gpsimd
    pe = nc.tensor

    keep = ctx.enter_context(tc.tile_pool(name="keep", bufs=1))

    def _dump(name, tile_ap, n):
        if dbg is not None and name in dbg:
            g.dma_start(dbg[name][:], _ap(tile_ap[:], 0, [[1, n]]))

    # ---- long-lived tiles ----
    Gt = keep.tile([128, F], DT_GS, tag="G")
    St = keep.tile([128, F], DT_GS, tag="S")
    Yt = keep.tile([128, F], BF16, tag="Y")
    Kt = keep.tile([128, 8], F32, tag="K")
    WMt = keep.tile([128, 512], BF16, tag="WM")
    LD = keep.tile([128, F], F32, tag="LD")
    nb0 = keep.tile([128, F], BF16, tag="nb0")
    nb1 = keep.tile([128, F], BF16, tag="nb1")
    pg = keep.tile([128, F], DT_PRED, tag="pg")
    pm = keep.tile([128, F], DT_PRED, tag="pm")
    AL = keep.tile([128, 2 * F], DT_AL, tag="AL")

    # ================= phase B pool (tree + loops), opened first ==========
    esB = ExitStack()
    tp = esB.enter_context(tc.tile_pool(name="tree", bufs=1))
    M = tp.tile([128, 4 * F], DT_TREE, tag="M")

    # ================= phase A pool (inputs + obs) ========================
    esA = ExitStack()
    pa = esA.enter_context(tc.tile_pool(name="obs", bufs=1))
    Ct = pa.tile([128, F], BF16, tag="C")
    for dram, sb_t in ((C, Ct), (G, Gt), (S, St), (Y, Yt), (K, Kt), (WM, WMt)):
        sy.dma_start(sb_t[:], dram[:])

    c2m1 = pa.tile([128, F], DT_TREE, tag="c2m1")
    ag = pa.tile([128, F], DT_TREE, tag="ag")
    as_ = pa.tile([128, F], DT_TREE, tag="as")
    u0 = pa.tile([128, F], DT_TREE, tag="u0")
    u1 = pa.tile([128, F], DT_TREE, tag="u1")
    v.tensor_scalar(c2m1[:], Ct[:], 2.0, -1.0, Alu.mult, Alu.add)
    v.tensor_tensor(ag[:], c2m1[:], Gt[:], op=Alu.mult)
    v.tensor_tensor(as_[:], c2m1[:], St[:], op=Alu.mult)
    sc.activation(u0[:], ag[:], Act.Sigmoid)
    sc.activation(u1[:], as_[:], Act.Sigmoid, scale=-1.0)
    sc.activation(pg[:], Gt[:], Act.Sigmoid)
    sc.activation(pm[:], St[:], Act.Sigmoid, scale=-1.0)

    # ---- M build: entry e at e*F;  M[c,m] = A[m,c]*u_m, e = 2c+m ----
    for (e, uu, kc) in ((0, u0, 0), (1, u1, 2), (2, u0, 1), (3, u1, 3)):
        v.tensor_scalar_mul(_ap(M[:], e * F, [[1, F]]), uu[:], Kt[:, kc:kc + 1])
    _dump("D_M", M, 4 * F)
    esA.close()

    # ---- product tree ----
    t1s = tp.tile([128, 4 * 768], DT_TREE, tag="t1")
    t2s = tp.tile([128, 4 * 768], DT_TREE, tag="t2")

    def combine(dst, X, w, npair):
        """X: [4, 2*npair, w] (entry-major, pair-slab-major) -> dst [4, npair, w]."""
        ei = 2 * npair * w
        eo = npair * w
        pd = [[2 * w, npair], [1, w]] if npair > 1 else [[1, w]]
        po = [[w, npair], [1, w]] if npair > 1 else [[1, w]]
        for i in (0, 1):
            v.tensor_tensor(
                _ap(t1s[:], 2 * i * eo, [[eo, 2]] + po),
                _ap(X[:], 2 * i * ei + w, [[0, 2]] + pd),
                _ap(X[:], 0, [[ei, 2]] + pd), op=Alu.mult)
            v.tensor_tensor(
                _ap(t2s[:], 2 * i * eo, [[eo, 2]] + po),
                _ap(X[:], (2 * i + 1) * ei + w, [[0, 2]] + pd),
                _ap(X[:], 2 * ei, [[ei, 2]] + pd), op=Alu.mult)
        v.tensor_tensor(_ap(dst[:], 0, [[1, 4 * eo]]),
                        _ap(t1s[:], 0, [[1, 4 * eo]]),
                        _ap(t2s[:], 0, [[1, 4 * eo]]), op=Alu.add)

    U2 = tp.tile([128, 4 * 768], DT_TREE, tag="U2")
    U4 = tp.tile([128, 4 * 384], DT_TREE, tag="U4")
    Q = tp.tile([128, 4 * NRB], DT_TREE, tag="Q")
    combine(U2, M, NRB, 4)
    combine(U4, U2, NRB, 2)
    combine(Q, U4, NRB, 1)

    # normalize block mats
    s01 = tp.tile([128, NRB], DT_TREE, tag="s01")
    Zb = tp.tile([128, NRB], DT_TREE, tag="Zb")
    rz = tp.tile([128, NRB], F32, tag="rz")
    v.tensor_tensor(s01[:], _ap(Q[:], 0, [[1, NRB]]),
                    _ap(Q[:], NRB, [[1, NRB]]), op=Alu.add)
    v.tensor_tensor(Zb[:], _ap(Q[:], 2 * NRB, [[1, NRB]]),
                    _ap(Q[:], 3 * NRB, [[1, NRB]]), op=Alu.add)
    v.tensor_tensor(Zb[:], s01[:], Zb[:], op=Alu.add)
    v.reciprocal(rz[:], Zb[:])
    rzc = rz
    if DT_TREE != F32:
        rzc = tp.tile([128, NRB], DT_TREE, tag="rzc")
        v.tensor_copy(rzc[:], rz[:])
    Qn = tp.tile([128, 4 * NRB], DT_TREE, tag="Qn")
    v.tensor_tensor(_ap(Qn[:], 0, [[NRB, 4], [1, NRB]]),
                    _ap(Q[:], 0, [[NRB, 4], [1, NRB]]),
                    _ap(rzc[:], 0, [[0, 4], [1, NRB]]), op=Alu.mult)

    _dump("D_Q", Qn, 4 * NRB)
    # block-level tree: pairs over jb (stride NS=24)
    U2b = tp.tile([128, 4 * 96], DT_TREE, tag="U2b")
    U4b = tp.tile([128, 4 * 48], DT_TREE, tag="U4b")
    SBm = tp.tile([128, 4 * NS], DT_TREE, tag="SBm")
    combine(U2b, Qn, NS, 4)
    combine(U4b, U2b, NS, 2)
    combine(SBm, U4b, NS, 1)

    _dump("D_SB", SBm, 4 * NS)
    # ---- super-level: sequential alpha over sb within each r (fp32) ----
    AS = tp.tile([128, 2 * NS], F32, tag="AS")
    tmps = tp.tile([128, 12], F32, tag="tmps")
    for m in (0, 1):
        v.tensor_scalar_mul(_ap(AS[:], m * NS, [[8, RT]]),
                            _ap(Kt[:], 4 + m, [[0, RT]]), 1.0)
    for sb in range(1, SBN):
        v.tensor_tensor(
            _ap(tmps[:], 0, [[6, 2], [3, 2], [1, RT]]),
            _ap(SBm[:], sb - 1, [[2 * NS, 2], [NS, 2], [8, RT]]),
            _ap(AS[:], sb - 1, [[0, 2], [NS, 2], [8, RT]]), op=Alu.mult)
        v.tensor_tensor(
            _ap(AS[:], sb, [[NS, 2], [8, RT]]),
            _ap(tmps[:], 0, [[6, 2], [1, RT]]),
            _ap(tmps[:], 3, [[6, 2], [1, RT]]), op=Alu.add)
    AZ = tp.tile([128, NS], F32, tag="AZ")
    v.tensor_tensor(AZ[:], _ap(AS[:], 0, [[1, NS]]),
                    _ap(AS[:], NS, [[1, NS]]), op=Alu.add)
    v.reciprocal(AZ[:], AZ[:])
    ASn = tp.tile([128, 2 * NS], DT_AL, tag="ASn")
    v.tensor_tensor(_ap(ASn[:], 0, [[NS, 2], [1, NS]]),
                    _ap(AS[:], 0, [[NS, 2], [1, NS]]),
                    _ap(AZ[:], 0, [[0, 2], [1, NS]]), op=Alu.mult)

    # ---- jb-loop: alpha at block starts  AB[m*NRB + jb*NS + s] ----
    AB = tp.tile([128, 2 * NRB], DT_AL, tag="AB")
    tmp4 = tp.tile([128, 4 * NS], DT_AL, tag="tmp4")
    v.tensor_copy(_ap(AB[:], 0, [[NRB, 2], [1, NS]]),
                  _ap(ASn[:], 0, [[NS, 2], [1, NS]]))
    for jb in range(1, SBN):
        v.tensor_tensor(
            _ap(tmp4[:], 0, [[2 * NS, 2], [NS, 2], [1, NS]]),
            _ap(Qn[:], (jb - 1) * NS, [[2 * NRB, 2], [NRB, 2], [1, NS]]),
            _ap(AB[:], (jb - 1) * NS, [[0, 2], [NRB, 2], [1, NS]]),
            op=Alu.mult)
        v.tensor_tensor(
            _ap(AB[:], jb * NS, [[NRB, 2], [1, NS]]),
            _ap(tmp4[:], 0, [[2 * NS, 2], [1, NS]]),
            _ap(tmp4[:], NS, [[2 * NS, 2], [1, NS]]), op=Alu.add)

    _dump("D_AB", AB, 2 * NRB)
    # ---- j-loop: alpha at all t  AL[m*F + j*NRB + rb] ----
    tmpd = tp.tile([128, 4 * NRB], DT_AL, tag="tmpd")
    v.tensor_copy(_ap(AL[:], 0, [[F, 2], [1, NRB]]),
                  _ap(AB[:], 0, [[NRB, 2], [1, NRB]]))
    for j in range(1, J):
        v.tensor_tensor(
            _ap(tmpd[:], 0, [[2 * NRB, 2], [NRB, 2], [1, NRB]]),
            _ap(M[:], (j - 1) * NRB, [[2 * F, 2], [F, 2], [1, NRB]]),
            _ap(AL[:], (j - 1) * NRB, [[0, 2], [F, 2], [1, NRB]]),
            op=Alu.mult)
        v.tensor_tensor(
            _ap(AL[:], j * NRB, [[F, 2], [1, NRB]]),
            _ap(tmpd[:], 0, [[2 * NRB, 2], [1, NRB]]),
            _ap(tmpd[:], NRB, [[2 * NRB, 2], [1, NRB]]), op=Alu.add)
    _dump("D_AL", AL, 2 * F)
    esB.close()

    # ================= phase C pool: predictions + lp + cumsum ============
    esC = ExitStack()
    pp = esC.enter_context(tc.tile_pool(name="pred", bufs=1))
    num1 = pp.tile([128, F], DT_PRED, tag="num1")
    num0 = pp.tile([128, F], DT_PRED, tag="num0")
    den = pp.tile([128, F], DT_PRED, tag="den")
    tt1 = pp.tile([128, F], DT_PRED, tag="tt1")
    tt2 = pp.tile([128, F], DT_PRED, tag="tt2")
    al0 = _ap(AL[:], 0, [[1, F]])
    al1 = _ap(AL[:], F, [[1, F]])
    v.tensor_tensor(tt1[:], al0, pg[:], op=Alu.mult)
    v.tensor_tensor(tt2[:], al1, pm[:], op=Alu.mult)
    v.tensor_tensor(num1[:], tt1[:], tt2[:], op=Alu.add)
    g.tensor_tensor(den[:], al0, al1, op=Alu.add)

    # ---- Ln outputs per slab (contiguous jbm views) ----
    jbm4 = [[8, RT], [1, SBN], [NS, SBN], [NRB, J]]       # (r, sb, jb, j)
    tord4 = [[T, RT], [64, SBN], [8, SBN], [1, J]]
    L1t = pp.tile([128, F], F32, tag="L1t")
    L0t = pp.tile([128, F], F32, tag="L0t")
    jbs = [[1, SBN], [NS, SBN], [NRB, J]]
    for r in range(RT):
        sv = lambda t_: _ap(t_[:], 8 * r, jbs)
        v.tensor_tensor(sv(num0), sv(den), sv(num1), op=Alu.subtract)
        sc.activation(sv(L1t), sv(num1), Act.Ln)
        sc.activation(sv(L0t), sv(num0), Act.Ln)
        sc.activation(sv(LD), sv(den), Act.Ln)

    # ---- lp (jbm) slab-chunked; -LD fused into the t-order relayout ----
    dL = pp.tile([128, F], F32, tag="dL")
    lpt = pp.tile([128, F], F32, tag="lpt")
    LPt = pp.tile([128, RT * T], F32, tag="LPt")
    APt = keep.tile([128, RT * (T + 1)], F32, tag="APt")
    g.memset(_ap(APt[:], 0, [[T + 1, RT]]), 0.0)
    jbm1 = [[1, SBN], [NS, SBN], [NRB, J]]      # one slab, t-order iter
    for r in range(RT):
        eng = g if r == 2 else v
        sl_ = lambda t_: _ap(t_[:], 8 * r, jbm1)
        eng.tensor_tensor(sl_(dL), sl_(L1t), sl_(L0t), op=Alu.subtract)
        eng.tensor_tensor(sl_(dL), sl_(Yt), sl_(dL), op=Alu.mult)
        eng.tensor_tensor(sl_(lpt), sl_(dL), sl_(L0t), op=Alu.add)
        eng.tensor_tensor(_ap(LPt[:], r * T, [[1, T]]), sl_(lpt), sl_(LD),
                          op=Alu.subtract)
        v.tensor_tensor_scan(_ap(APt[:], r * (T + 1) + 1, [[1, T]]),
                             _ap(LPt[:], r * T, [[1, T]]),
                             _ap(LPt[:], r * T, [[1, T]]),
                             0.0, Alu.add, Alu.bypass)
    # nb copies (Pool; consumed by EX much later)
    g.tensor_copy(_ap(nb0[:], 0, tord4), _ap(num0[:], 0, jbm4))
    g.tensor_copy(_ap(nb1[:], 0, tord4), _ap(num1[:], 0, jbm4))
    # pad slab (r=2, hi partitions): force ap to -inf-ish (t>=1)
    g.memset(_ap_p(APt[:], 64, 64, 2 * (T + 1) + 1, [[1, T]]), -1e9)
    _dump("D_AP", APt, RT * (T + 1))
    esC.close()

    # ================= phase D pool: collapse =============================
    esD = ExitStack()
    cp = esD.enter_context(tc.tile_pool(name="col", bufs=1))
    apslab = lambda r: _ap(APt[:], r * (T + 1), [[1, T]])
    M1 = cp.tile([128, 512], F32, tag="M1")
    MO = cp.tile([128, 512], F32, tag="MO")
    v.tensor_tensor(M1[:], apslab(0), apslab(1), op=Alu.max)
    sc.copy(_ap_p(MO[:], 64, 64, 0, [[1, 512]]),
            _ap_p(M1[:], 64, 64, 0, [[1, 512]]))
    v.tensor_tensor(_ap_p(MO[:], 0, 64, 0, [[1, 512]]),
                    _ap_p(M1[:], 0, 64, 0, [[1, 512]]),
                    bass.AP(APt[:].tensor, APt[:].offset + 2 * (T + 1),
                            [[list(APt[:].ap[0])[0], 64], [1, T]]),
                    op=Alu.max)
    MSW = cp.tile([64, 512], F32, tag="MSW")
    sy.dma_start(MSW[:], _ap_p(MO[:], 64, 64, 0, [[1, 512]]))
    Mx = cp.tile([64, 512], F32, tag="Mx")
    dA = cp.tile([64, 512], F32, tag="dA")
    dB = cp.tile([64, 512], F32, tag="dB")
    eA = cp.tile([64, 512], F32, tag="eA")
    eB = cp.tile([64, 512], F32, tag="eB")
    mo_lo = _ap_p(MO[:], 0, 64, 0, [[1, 512]])
    v.tensor_tensor(Mx[:], mo_lo, MSW[:], op=Alu.max)
    v.tensor_tensor(dA[:], mo_lo, Mx[:], op=Alu.subtract)
    v.tensor_tensor(dB[:], MSW[:], Mx[:], op=Alu.subtract)
    sc.activation(eA[:], dA[:], Act.Exp)
    sc.activation(eB[:], dB[:], Act.Exp)

    # ---- E = exp(ap - m - LD);  EX_k = num_k * E  (per-slab pipeline) ----
    Wt = cp.tile([128, F], F32, tag="Wt")
    E = cp.tile([128, F], BF16, tag="E")
    EX0 = cp.tile([128, F], BF16, tag="EX0")
    EX1 = cp.tile([128, F], BF16, tag="EX1")
    for r in range(RT):
        wsl = _ap(Wt[:], r * T, [[1, T]])
        v.tensor_tensor(wsl, apslab(r), MO[:], op=Alu.subtract)
        v.tensor_tensor(wsl, wsl,
                        _ap(LD[:], 8 * r, [[1, SBN], [NS, SBN], [NRB, J]]),
                        op=Alu.subtract)
        sc.activation(_ap(E[:], r * T, [[1, T]]), wsl, Act.Exp)
        v.tensor_tensor(_ap(EX0[:], r * T, [[1, T]]),
                        _ap(nb0[:], r * T, [[1, T]]),
                        _ap(E[:], r * T, [[1, T]]), op=Alu.mult)
        v.tensor_tensor(_ap(EX1[:], r * T, [[1, T]]),
                        _ap(nb1[:], r * T, [[1, T]]),
                        _ap(E[:], r * T, [[1, T]]), op=Alu.mult)

    # ---- PE collapse: 4 banks, k sums all landing on partitions 0-63 ----
    ps = ctx.enter_context(tc.tile_pool(name="ps", bufs=1, space="PSUM"))
    bank1 = ps.tile([128, 512], F32, tag="bank1")   # S_lo_k0
    bank2 = ps.tile([128, 512], F32, tag="bank2")   # S_hi_k0
    bank3 = ps.tile([128, 512], F32, tag="bank3")   # S_lo_k1
    bank4 = ps.tile([128, 512], F32, tag="bank4")   # S_hi_k1
    tslab = lambda t_, r: _ap(t_[:], r * T, [[1, T]])
    wmask = lambda i: _ap(WMt[:], 128 * i, [[1, 128]])
    sched = [
        (bank1, 0, EX0, 0, "s"), (bank2, 2, EX0, 0, "s"),
        (bank3, 0, EX1, 0, "s"), (bank4, 2, EX1, 0, "s"),
        (bank1, 0, EX0, 1, ""), (bank2, 2, EX0, 1, "e"),
        (bank1, 0, EX0, 2, "e"), (bank3, 0, EX1, 1, ""),
        (bank4, 2, EX1, 1, "e"), (bank3, 0, EX1, 2, "e"),
    ]
    for bank, mi, ex, r, fl in sched:
        pe.matmul(bank[:], wmask(mi), tslab(ex, r),
                  start=("s" in fl), stop=("e" in fl), skip_group_check=True)

    # ---- un + output tail (all on partitions 0-63), chunked in halves ----
    ue0 = cp.tile([64, 512], F32, tag="ue0")
    ue1 = cp.tile([64, 512], F32, tag="ue1")
    tm0 = cp.tile([64, 512], F32, tag="tm0")
    un0 = cp.tile([64, 512], F32, tag="un0")
    un1 = cp.tile([64, 512], F32, tag="un1")
    dl = cp.tile([64, 512], F32, tag="dl")
    ed = cp.tile([64, 512], F32, tag="ed")
    spt = cp.tile([64, 512], F32, tag="spt")
    OI = cp.tile([64, 1024], F32, tag="OI")
    _dump("D_MO", MO, 512)
    H = 256
    for h in (0, 1):
        o = h * H
        hp = lambda t_: _ap(t_[:], o, [[1, H]])
        bk = lambda b_: _ap_p(b_[:], 0, 64, o, [[1, H]])
        v.tensor_tensor(hp(ue0), bk(bank1), hp(eA), op=Alu.mult)
        v.tensor_tensor(hp(tm0), bk(bank2), hp(eB), op=Alu.mult)
        v.tensor_tensor(hp(ue0), hp(ue0), hp(tm0), op=Alu.add)
        sc.activation(hp(un0), hp(ue0), Act.Ln)
        v.tensor_tensor(hp(ue1), bk(bank3), hp(eA), op=Alu.mult)
        v.tensor_tensor(hp(tm0), bk(bank4), hp(eB), op=Alu.mult)
        v.tensor_tensor(hp(ue1), hp(ue1), hp(tm0), op=Alu.add)
        sc.activation(hp(un1), hp(ue1), Act.Ln)
        v.tensor_tensor(hp(dl), hp(un0), hp(un1), op=Alu.subtract)
        sc.activation(hp(ed), hp(dl), Act.Exp)
        sc.activation(hp(spt), hp(ed), Act.Ln, bias=1.0)
        v.tensor_scalar_mul(_ap(OI[:], 2 * o + 1, [[2, H]]), hp(spt), -1.0)
        v.tensor_tensor(_ap(OI[:], 2 * o, [[2, H]]), hp(dl), hp(spt),
                        op=Alu.subtract)
        sy.dma_start(bass.AP(O[:].tensor, 2 * o, [[1024, BL], [1, 2 * H]]),
                     _ap(OI[:], 2 * o, [[1, 2 * H]]))
    _dump("D_UE", ue0, 512)
    esD.close()


def _steer_act_tables(arch):
    from concourse import hw_specs
    tabs = hw_specs.get_activation_tables(arch)
    for name, funcs in tabs.items():
        if name == "natural_log_exp_and_others":
            continue
        funcs.discard(Act.Exp)
        funcs.discard(Act.Ln)


def _build_program():
    nc = bacc.Bacc()
    _steer_act_tables(nc.m.arch)
    G = nc.declare_dram_parameter("G", [128, F], DT_GS, isOutput=False)
    S = nc.declare_dram_parameter("S", [128, F], DT_GS, isOutput=False)
    C = nc.declare_dram_parameter("C", [128, F], BF16, isOutput=False)
    Y = nc.declare_dram_parameter("Y", [128, F], BF16, isOutput=False)
    K = nc.declare_dram_parameter("K", [128, 8], F32, isOutput=False)
    WM = nc.declare_dram_parameter("WM", [128, 512], BF16, isOutput=False)
    O = nc.declare_dram_parameter("O", [BL, 1024], F32, isOutput=True)
    dbg = None
    if DEBUG:
        dbg = {n: nc.declare_dram_parameter(n, [128, sz], F32, isOutput=True)
               for n, sz in _DBG_SHAPES.items()}
    with ExitStack() as ctx:
        tc = ctx.enter_context(tile.TileContext(nc))
        _emit(ctx, tc, nc, G, S, C, Y, K, WM, O, dbg)
    if not nc.is_finalized():
        nc.finalize()
    return nc


def _perm(x_abt):
    """(5, 64, 512) -> (128, 1536) j-major core layout."""
    x5 = x_abt.reshape(5, 64, 8, 8, 8)                 # a, b, sb, jb, j
    lo = np.transpose(x5[0:3], (1, 4, 3, 0, 2))        # b, j, jb, r, sb
    hi = np.transpose(x5[3:5], (1, 4, 3, 0, 2))
    hi = np.concatenate(
        [hi, np.zeros((64, 8, 8, 1, 8), x5.dtype)], axis=3)
    return np.concatenate([lo, hi], axis=0).reshape(128, F)


def _perm_tord(x_abt):
    """(5, 64, 512) -> (128, 1536) t-order slabs: [r*512 + t]."""
    lo = np.transpose(x_abt[0:3], (1, 0, 2))
    hi = np.transpose(x_abt[3:5], (1, 0, 2))
    hi = np.concatenate([hi, np.zeros((64, 1, 512), x_abt.dtype)], axis=1)
    return np.concatenate([lo, hi], axis=0).reshape(128, F)


def _masks():
    wm = np.zeros((128, 512), dtype=np.float32)
    i = np.arange(64)
    wm[i, 0 * 128 + i] = 1.0          # bank1 k0: lo -> out p
    wm[64 + i, 1 * 128 + 64 + i] = 1.0  # bank1 k1: hi -> out 64+p
    wm[64 + i, 2 * 128 + i] = 1.0     # bank2 k0: hi -> out p
    wm[i, 3 * 128 + 64 + i] = 1.0     # bank2 k1: lo -> out 64+p
    return wm


def kernel(corr, ytrue, problem, kc, dyn_emb, obs_logits_problem,
           obs_logits_kc, ability_levels, traj, trans_ind, pred_ind):
    global _last_results, _cached_nc
    import ml_dtypes
    bf16 = ml_dtypes.bfloat16

    corr = np.asarray(corr, dtype=np.float32)
    ytrue = np.asarray(ytrue, dtype=np.float32)
    problem = np.asarray(problem)
    kc = np.asarray(kc)
    dyn_emb = np.asarray(dyn_emb, dtype=np.float32)
    olp = np.asarray(obs_logits_problem, dtype=np.float32)
    olk = np.asarray(obs_logits_kc, dtype=np.float32)
    ability = np.asarray(ability_levels, dtype=np.float32)

    sig = lambda x: 1.0 / (1.0 + np.exp(-x.astype(np.float64)))
    dyn = dyn_emb[kc]
    lL, lF, lI0 = dyn[:, 0], dyn[:, 1], dyn[:, 2]
    Kfull = np.stack(
        [sig(-lL), sig(lL), sig(lF), sig(-lF), sig(-lI0), sig(lI0),
         np.zeros_like(lL), np.zeros_like(lL)], axis=1
    ).astype(np.float32)                                  # (512, 8)

    wm = _masks().astype(bf16)
    np_gs = np.float32 if DT_GS == F32 else bf16

    in_maps = []
    for c in range(N_CORES):
        sl = slice(c * BL, (c + 1) * BL)
        obs = olp[problem[sl]] + olk[kc[sl]][:, None, :]  # (64, 512, 2)
        g5 = obs[None, :, :, 0] + ability[:, None, None]
        s5 = obs[None, :, :, 1] - ability[:, None, None]
        c5 = np.broadcast_to(corr[sl][None], (5, BL, T))
        y5 = np.broadcast_to(ytrue[sl][None], (5, BL, T))
        in_maps.append({
            "G": np.ascontiguousarray(_perm(g5.astype(np.float32))).astype(np_gs),
            "S": np.ascontiguousarray(_perm(s5.astype(np.float32))).astype(np_gs),
            "C": np.ascontiguousarray(_perm(np.ascontiguousarray(c5))).astype(bf16),
            "Y": np.ascontiguousarray(_perm(np.ascontiguousarray(y5))).astype(bf16),
            "K": np.tile(Kfull[sl], (2, 1)),
            "WM": wm,
        })

    if _cached_nc is None:
        _cached_nc = _build_program()

    res = run_bass_kernel_spmd(
        _cached_nc, in_maps, list(range(N_CORES)),
        trace=bool(os.environ.get("BASS_TRACE")),
    )
    _last_results = res
    out = np.concatenate(
        [res.results[i]["O"].reshape(BL, T, 2) for i in range(N_CORES)],
        axis=0)
    return out.astype(np.float32)
